# revision 58
# baseline (speedup 1.0000x reference)
"""Trainium2 Bass kernel for multi-head self-attention (B=2, N=2048, DIM=1024,
16 heads x 64). Sharding: core i handles batch b=i//4 and 4 heads hg=i%4
(tensor-parallel on heads: column-shard Wq/Wkv, row-shard Wo; partial outputs
summed on host).

Math notes:
  - `similarity` adds a per-query constant along the softmax axis, so softmax
    is invariant to it -> it is accepted but unused.
  - Softmax computed without max-subtraction (logits are O(10); exp is safe in
    fp32) as exp(dots)/Z with Z obtained for free as a 65th "ones" column of V
    in the E@V matmul.
  - Everything is computed transposed (q^T, k^T laid out [d, n]) so no
    on-device transposes are needed anywhere.
"""

import os
import sys

import numpy as np

sys.path.insert(0, "/opt/trn_rl_repo")

import ml_dtypes

B, N, DIM = 2, 2048, 1024
HEADS, DHEAD = 16, 64
HG = 4  # heads per core
SCALE = DHEAD**-0.5
NCORES = 8
P = 128
NI = 512  # i-chunk (matmul moving free dim)
NIC = N // NI  # 4 i-chunks
NJT = N // P  # 16 j tiles
CT = DIM // P  # 8 contraction tiles
GJ = 3  # j-tiles per dots psum staging group

LAST_RESULTS = None
_CACHED_NC = None


def _ensure_profile_hook():
    """Provide antenv.axon_hooks (absent in this image) so that
    run_bass_kernel_spmd(trace=True) can NTFF-profile through axon."""
    import contextlib
    import ctypes
    import types

    try:
        import antenv.axon_hooks  # noqa: F401

        return
    except ImportError:
        pass
    if "antenv.axon_hooks" in sys.modules:
        return
    mod = types.ModuleType("antenv.axon_hooks")
    state = {"hook": None}
    mod.set_axon_ntff_profile_hook = lambda h: state.__setitem__("hook", h)
    mod.get_axon_ntff_profile_hook = lambda: state["hook"]
    sys.modules["antenv.axon_hooks"] = mod
    try:
        import antenv

        antenv.axon_hooks = mod
    except ImportError:
        pass

    so_path = "/opt/axon/libaxon_pjrt.so"
    if not os.path.exists(so_path):
        return
    try:
        lib = ctypes.CDLL(so_path)
    except OSError:
        return
    if not hasattr(lib, "axon_start_nrt_profile"):
        return
    lib.axon_start_nrt_profile.argtypes = [
        ctypes.POINTER(ctypes.c_int64),
        ctypes.c_size_t,
    ]
    lib.axon_start_nrt_profile.restype = ctypes.c_int64
    lib.axon_stop_nrt_profile.argtypes = [ctypes.c_char_p]
    lib.axon_stop_nrt_profile.restype = ctypes.c_int64

    @contextlib.contextmanager
    def _hook(output_dir, device_ids):
        import jax

        jax.devices()
        if device_ids:
            ids = (ctypes.c_int64 * len(device_ids))(*device_ids)
            rc = lib.axon_start_nrt_profile(ids, len(device_ids))
        else:
            rc = lib.axon_start_nrt_profile(None, 0)
        if rc != 0:
            raise RuntimeError(f"axon_start_nrt_profile rc={rc}")
        try:
            yield
        finally:
            n = lib.axon_stop_nrt_profile(str(output_dir).encode())
            print(f"ntff profile: {n} file(s) written to {output_dir}")

    mod.set_axon_ntff_profile_hook(_hook)


def _build_program():
    import concourse.tile as tile
    from concourse import bacc, mybir

    f32 = mybir.dt.float32
    bf16 = mybir.dt.bfloat16
    Exp = mybir.ActivationFunctionType.Exp

    nc = bacc.Bacc("TRN2", target_bir_lowering=False, debug=False)
    xT = nc.dram_tensor("xT", [DIM, N], bf16, kind="ExternalInput").ap()
    wq = nc.dram_tensor("wq", [DIM, HG * DHEAD], bf16, kind="ExternalInput").ap()
    wk = nc.dram_tensor("wk", [DIM, HG * DHEAD], bf16, kind="ExternalInput").ap()
    wv = nc.dram_tensor("wv", [DIM, HG * DHEAD], bf16, kind="ExternalInput").ap()
    wo = nc.dram_tensor("wo", [HG * DHEAD, DIM], bf16, kind="ExternalInput").ap()
    out = nc.dram_tensor("out", [N, DIM], f32, kind="ExternalOutput").ap()

    with tile.TileContext(nc) as tc:
        _emit(tc, nc, mybir, out, xT, wq, wk, wv, wo, f32, bf16, Exp)
    nc.compile()
    return nc


def _emit(tc, nc, mybir, out, xT, wq, wk, wv, wo, f32, bf16, Exp):
    with (
        tc.tile_pool(name="cpool", bufs=1) as cpool,
        tc.tile_pool(name="ppool", bufs=4, space="PSUM") as ppool,
        tc.tile_pool(name="dpool", bufs=2, space="PSUM") as dpool,
        tc.tile_pool(name="epool", bufs=2) as epool,
        tc.tile_pool(name="wpool", bufs=2) as wpool,
        tc.tile_pool(name="opool", bufs=3) as opool,
        tc.tile_pool(name="drpool", bufs=2, space="DRAM") as drpool,
    ):
        # ---- constants. DMA order matters: V-projection work is first, so
        # wv and the first x^T column-chunk lead both queues; wq/wk/wo are
        # needed only ~35us in.
        wv_sb = cpool.tile([P, CT, 256], bf16, name="wv_sb")
        nc.sync.dma_start(wv_sb[:], wv.rearrange("(t p) m -> p t m", p=P))
        xt = cpool.tile([P, CT, N], bf16, name="xt")
        wq_sb = cpool.tile([P, CT, 256], bf16, name="wq_sb")
        wk_sb = cpool.tile([P, CT, 256], bf16, name="wk_sb")
        wo_sb = cpool.tile([P, 2, DIM], bf16, name="wo_sb")

        def _xt_cc(cc):
            for t in range(CT):
                eng = (nc.sync, nc.scalar)[t % 2]
                eng.dma_start(
                    xt[:, t, cc * NI : (cc + 1) * NI],
                    xT[t * P : (t + 1) * P, cc * NI : (cc + 1) * NI],
                )

        _xt_cc(0)
        nc.scalar.dma_start(wq_sb[:], wq.rearrange("(t p) m -> p t m", p=P))
        nc.scalar.dma_start(wk_sb[:], wk.rearrange("(t p) m -> p t m", p=P))
        nc.scalar.dma_start(wo_sb[:], wo.rearrange("(t p) m -> p t m", p=P))
        for cc in range(1, 4):
            _xt_cc(cc)

        # Q^T, K^T [256, N] as 2 partition-tiles; V padded to 128 cols per
        # head: [v(64) | ones(1) | zeros(63)] so lhsT is 128 wide (FWL).
        QT = cpool.tile([P, 2, N], bf16, name="QT")
        KT = cpool.tile([P, 2, N], bf16, name="KT")
        Vo = cpool.tile([P, NJT, HG * 65], bf16, name="Vo")
        Vo_heads = Vo.rearrange("p j (h c) -> p j h c", c=65)
        nc.vector.memset(Vo_heads[:, :, :, 64:65], 1.0)
        ones_sb = cpool.tile([P, 64], bf16, name="ones_sb")
        nc.vector.memset(ones_sb[:], 1.0)

        # ---- emission helpers ----
        def emit_proj_qk(pt, ic):
            q_ps = ppool.tile([P, NI], f32, tag="acc", name="q_ps")
            for ct in range(CT):
                nc.tensor.matmul(
                    q_ps,
                    lhsT=wq_sb[:, ct, pt * P : (pt + 1) * P],
                    rhs=xt[:, ct, ic * NI : (ic + 1) * NI],
                    start=(ct == 0),
                    stop=(ct == CT - 1),
                )
            nc.vector.tensor_copy(out=QT[:, pt, ic * NI : (ic + 1) * NI], in_=q_ps)
            k_ps = ppool.tile([P, NI], f32, tag="acc", name="k_ps")
            for ct in range(CT):
                nc.tensor.matmul(
                    k_ps,
                    lhsT=wk_sb[:, ct, pt * P : (pt + 1) * P],
                    rhs=xt[:, ct, ic * NI : (ic + 1) * NI],
                    start=(ct == 0),
                    stop=(ct == CT - 1),
                )
            nc.vector.tensor_copy(out=KT[:, pt, ic * NI : (ic + 1) * NI], in_=k_ps)

        def emit_proj_v(jt):
            v_ps = ppool.tile([P, 256], f32, tag="acc", name="v_ps")
            for ct in range(CT):
                nc.tensor.matmul(
                    v_ps,
                    lhsT=xt[:, ct, jt * P : (jt + 1) * P],
                    rhs=wv_sb[:, ct, :],
                    start=(ct == 0),
                    stop=(ct == CT - 1),
                )
            nc.vector.tensor_copy(
                out=Vo_heads[:, jt, :, 0:64],
                in_=v_ps.rearrange("p (h c) -> p h c", h=HG),
            )

        def emit_chunk_mms(pt, ic):
            """dots -> exp -> E@V accumulation for one (head-pair, i-chunk)."""
            hA, hB = 2 * pt, 2 * pt + 1
            E = epool.tile([P, 2, NJT, NI], bf16, tag="E", name="E")
            otA = ppool.tile([65, NI], f32, tag="acc", name="otA")
            otB = ppool.tile([65, NI], f32, tag="acc", name="otB")

            def emit_ot(jt):
                nc.tensor.matmul(
                    otA,
                    lhsT=Vo_heads[:, jt, hA, :],
                    rhs=E[:, 0, jt, :],
                    start=(jt == 0),
                    stop=(jt == NJT - 1),
                )
                nc.tensor.matmul(
                    otB,
                    lhsT=Vo_heads[:, jt, hB, :],
                    rhs=E[:, 1, jt, :],
                    start=(jt == 0),
                    stop=(jt == NJT - 1),
                )

            # software-pipelined: E@V for jt-1 is emitted after dots for jt,
            # so the PE never stalls on the exp of the tile it just produced
            for jt in range(NJT):
                dAB = dpool.tile([P, 2, NI], f32, tag="dAB", name="dAB")
                nc.tensor.matmul(
                    dAB[:, 0, :],
                    lhsT=KT[0:64, pt, jt * P : (jt + 1) * P],
                    rhs=QT[0:64, pt, ic * NI : (ic + 1) * NI],
                    start=True,
                    stop=True,
                )
                nc.tensor.matmul(
                    dAB[:, 1, :],
                    lhsT=KT[64:128, pt, jt * P : (jt + 1) * P],
                    rhs=QT[64:128, pt, ic * NI : (ic + 1) * NI],
                    start=True,
                    stop=True,
                )
                nc.scalar.activation(
                    out=E[:, :, jt, :], in_=dAB[:], func=Exp, scale=SCALE
                )
                if jt > 0:
                    emit_ot(jt - 1)
            emit_ot(NJT - 1)
            return otA, otB

        onp_tiles = {}

        def emit_norm_front(pt, ic, otA, otB):
            """DVE-only part: drain Z/O out of PSUM (frees acc slots) and
            compute 1/Z. Returns state for emit_norm_back."""
            zrow = wpool.tile([65, 2 * NI], f32, tag="zrow", name="zrow")
            zi32 = wpool.tile([65, 2 * NI], f32, tag="zi32", name="zi32")
            zi = wpool.tile([65, 2 * NI], bf16, tag="zi", name="zi")
            nc.vector.tensor_copy(out=zrow[64:65, 0:NI], in_=otA[64:65, :])
            tmpA = wpool.tile([64, NI], bf16, tag="tmp", name="tmpA", bufs=4)
            nc.vector.tensor_copy(out=tmpA, in_=otA[0:64, :])
            nc.vector.tensor_copy(out=zrow[64:65, NI : 2 * NI], in_=otB[64:65, :])
            tmpB = wpool.tile([64, NI], bf16, tag="tmp", name="tmpB", bufs=4)
            nc.vector.tensor_copy(out=tmpB, in_=otB[0:64, :])
            nc.vector.reciprocal(zi32[64:65, :], zrow[64:65, :])
            nc.vector.tensor_copy(out=zi[64:65, :], in_=zi32[64:65, :])
            # hop 1/Z to partition 0 (partition_broadcast reads partition 0)
            zi0 = wpool.tile([1, 2 * NI], bf16, tag="zi0", name="zi0", bufs=2)
            nc.sync.dma_start(zi0[:], zi[64:65, :])
            return (pt, ic, zi0, tmpA, tmpB)

        def emit_norm_back(st):
            """PE broadcast of 1/Z + the normalization multiplies."""
            pt, ic, zi0, tmpA, tmpB = st
            onp = wpool.tile([P, NI], bf16, tag=f"onp{pt}_{ic}", name="onp")
            zbA = wpool.tile([64, NI], bf16, tag="zbb", name="zbA", bufs=2)
            nc.gpsimd.partition_broadcast(zbA[:], zi0[0:1, 0:NI])
            nc.vector.tensor_mul(out=onp[0:64, :], in0=tmpA, in1=zbA)
            zbB = wpool.tile([64, NI], bf16, tag="zbb", name="zbB", bufs=2)
            nc.gpsimd.partition_broadcast(zbB[:], zi0[0:1, NI : 2 * NI])
            nbuf = wpool.tile([64, NI], bf16, tag="nbuf", name="nbuf")
            nc.vector.tensor_mul(out=nbuf, in0=tmpB, in1=zbB)
            # cross-partition move (rows 0-63 -> 64-127) via DMA
            nc.sync.dma_start(onp[64:128, :], nbuf)
            onp_tiles[(pt, ic)] = onp

        def emit_outproj(ic):
            for it_in in range(NI // P):
                for ec in range(2):
                    o_ps = ppool.tile([P, NI], f32, tag="acc", name="o_ps")
                    for pt in range(2):
                        nc.tensor.matmul(
                            o_ps,
                            lhsT=onp_tiles[(pt, ic)][:, it_in * P : (it_in + 1) * P],
                            rhs=wo_sb[:, pt, ec * NI : (ec + 1) * NI],
                            start=(pt == 0),
                            stop=(pt == 1),
                        )
                    osb = opool.tile([P, NI], f32, tag="osb", name="osb")
                    nc.vector.tensor_copy(out=osb, in_=o_ps)
                    it = ic * (NI // P) + it_in
                    nc.sync.dma_start(
                        out[it * P : (it + 1) * P, ec * NI : (ec + 1) * NI], osb
                    )

        # ---- schedule ----
        # V and pair-0 Q/K first; pair-1 Q/K interleaved between pair-0
        # chunks; norm deferred one chunk so PE never waits on the (DVE/DMA)
        # normalization chain; outproj(ic) once both pairs of ic are normed.
        for jt in range(NJT):
            emit_proj_v(jt)
        for ic in range(NIC):
            emit_proj_qk(0, ic)

        chunks = [(0, ic) for ic in range(NIC)] + [(1, ic) for ic in range(NIC)]
        pending = None  # (pt, ic, otA, otB) awaiting norm front
        back_queue = []  # norm states awaiting the zb/mul back-half
        outproj_queue = []  # ics with both pairs normed, outproj not emitted
        for n, (pt, ic) in enumerate(chunks):
            otA, otB = emit_chunk_mms(pt, ic)
            if n < NIC:
                emit_proj_qk(1, n)
            if pending is not None:
                ppt, pic, pA, pB = pending
                st = emit_norm_front(ppt, pic, pA, pB)
                # back follows its front immediately; the whole chain then
                # completes during the next chunk, and outproj (one chunk
                # later) never waits on it
                emit_norm_back(st)
                if st[0] == 1:
                    outproj_queue.append(st[1])
            while len(outproj_queue) > 1:
                emit_outproj(outproj_queue.pop(0))
            pending = (pt, ic, otA, otB)
        ppt, pic, pA, pB = pending
        st = emit_norm_front(ppt, pic, pA, pB)
        emit_norm_back(st)
        if st[0] == 1:
            outproj_queue.append(st[1])
        while outproj_queue:
            emit_outproj(outproj_queue.pop(0))


def _get_program():
    global _CACHED_NC
    if _CACHED_NC is None:
        _CACHED_NC = _build_program()
    return _CACHED_NC


def _shard_inputs(x, Wq, Wkv, Wo):
    bf = ml_dtypes.bfloat16
    xTs = [np.ascontiguousarray(x[b].T).astype(bf) for b in range(B)]
    wqs, wks, wvs, wos = [], [], [], []
    for hg in range(HG):
        c0 = hg * HG * DHEAD
        c1 = c0 + HG * DHEAD
        wqs.append(np.ascontiguousarray(Wq[:, c0:c1]).astype(bf))
        wks.append(np.ascontiguousarray(Wkv[:, c0:c1]).astype(bf))
        wvs.append(np.ascontiguousarray(Wkv[:, DIM + c0 : DIM + c1]).astype(bf))
        wos.append(np.ascontiguousarray(Wo[c0:c1, :]).astype(bf))
    in_maps = []
    for core in range(NCORES):
        b, hg = core // HG, core % HG
        in_maps.append(
            {
                "xT": xTs[b],
                "wq": wqs[hg],
                "wk": wks[hg],
                "wv": wvs[hg],
                "wo": wos[hg],
            }
        )
    return in_maps


def kernel(x, similarity, Wq, Wkv, Wo, bo):
    global LAST_RESULTS
    _ensure_profile_hook()
    import concourse.bass_utils as _bu
    from concourse.bass_utils import run_bass_kernel_spmd

    # keep trace artifacts local if profiling is ever enabled (no S3 here)
    _bu.upload_artifacts = lambda tmpdir: tmpdir

    x = np.asarray(x, dtype=np.float32)
    Wq = np.asarray(Wq, dtype=np.float32)
    Wkv = np.asarray(Wkv, dtype=np.float32)
    Wo = np.asarray(Wo, dtype=np.float32)
    bo = np.asarray(bo, dtype=np.float32)

    nc = _get_program()
    in_maps = _shard_inputs(x, Wq, Wkv, Wo)
    res = run_bass_kernel_spmd(nc, in_maps, list(range(NCORES)))
    LAST_RESULTS = res
    outs = [res.results[i]["out"] for i in range(NCORES)]
    full = np.empty((B, N, DIM), dtype=np.float32)
    for b in range(B):
        acc = outs[4 * b].astype(np.float32).copy()
        for hg in range(1, HG):
            acc += outs[4 * b + hg]
        full[b] = acc + bo[None, :]
    return full


def _sim_check():
    """Simulate core 0 on CoreSim and compare against numpy reference."""
    from concourse.bass_interp import CoreSim

    rng = np.random.default_rng(0)
    x = rng.standard_normal((B, N, DIM), dtype=np.float32)
    Wq = (rng.standard_normal((DIM, DIM), dtype=np.float32) * DIM**-0.5).astype(
        np.float32
    )
    Wkv = (
        rng.standard_normal((DIM, 2 * DIM), dtype=np.float32) * DIM**-0.5
    ).astype(np.float32)
    Wo = (rng.standard_normal((DIM, DIM), dtype=np.float32) * DIM**-0.5).astype(
        np.float32
    )

    nc = _get_program()
    in_maps = _shard_inputs(x, Wq, Wkv, Wo)
    core = 0
    sim = CoreSim(nc)
    for name, arr in in_maps[core].items():
        sim.tensor(name)[:] = arr
    sim.simulate()
    got = np.array(sim.tensor("out"))

    # numpy reference for core 0's partial (batch 0, heads 0-3), fp32 exact
    b, hg = 0, 0
    xb = x[b]
    q = xb @ Wq[:, hg * 256 : hg * 256 + 256]
    k = xb @ Wkv[:, hg * 256 : hg * 256 + 256]
    v = xb @ Wkv[:, DIM + hg * 256 : DIM + hg * 256 + 256]
    partial = np.zeros((N, DIM), dtype=np.float32)
    for h in range(HG):
        qh = q[:, h * 64 : h * 64 + 64]
        kh = k[:, h * 64 : h * 64 + 64]
        vh = v[:, h * 64 : h * 64 + 64]
        dots = (qh @ kh.T) * SCALE
        dots -= dots.max(axis=-1, keepdims=True)
        e = np.exp(dots)
        attn = e / e.sum(axis=-1, keepdims=True)
        partial += (attn @ vh) @ Wo[hg * 256 + h * 64 : hg * 256 + h * 64 + 64, :]

    err = np.abs(got - partial)
    scale = np.abs(partial).max()
    print("max abs err:", err.max(), "scale:", scale, "rel:", err.max() / scale)
    return err.max() / scale


if __name__ == "__main__":
    _sim_check()


# revision 59
# speedup vs baseline: 1.0188x; 1.0188x over previous
"""Trainium2 Bass kernel for multi-head self-attention (B=2, N=2048, DIM=1024,
16 heads x 64). Sharding: core i handles batch b=i//4 and 4 heads hg=i%4
(tensor-parallel on heads: column-shard Wq/Wkv, row-shard Wo; partial outputs
summed on host).

Math notes:
  - `similarity` adds a per-query constant along the softmax axis, so softmax
    is invariant to it -> it is accepted but unused.
  - Softmax computed without max-subtraction (logits are O(10); exp is safe in
    fp32) as exp(dots)/Z with Z obtained for free as a 65th "ones" column of V
    in the E@V matmul.
  - Everything is computed transposed (q^T, k^T laid out [d, n]) so no
    on-device transposes are needed anywhere.
"""

import os
import sys

import numpy as np

sys.path.insert(0, "/opt/trn_rl_repo")

import ml_dtypes

B, N, DIM = 2, 2048, 1024
HEADS, DHEAD = 16, 64
HG = 4  # heads per core
SCALE = DHEAD**-0.5
NCORES = 8
P = 128
NI = 512  # i-chunk (matmul moving free dim)
NIC = N // NI  # 4 i-chunks
NJT = N // P  # 16 j tiles
CT = DIM // P  # 8 contraction tiles
GJ = 3  # j-tiles per dots psum staging group

LAST_RESULTS = None
_CACHED_NC = None


def _ensure_profile_hook():
    """Provide antenv.axon_hooks (absent in this image) so that
    run_bass_kernel_spmd(trace=True) can NTFF-profile through axon."""
    import contextlib
    import ctypes
    import types

    try:
        import antenv.axon_hooks  # noqa: F401

        return
    except ImportError:
        pass
    if "antenv.axon_hooks" in sys.modules:
        return
    mod = types.ModuleType("antenv.axon_hooks")
    state = {"hook": None}
    mod.set_axon_ntff_profile_hook = lambda h: state.__setitem__("hook", h)
    mod.get_axon_ntff_profile_hook = lambda: state["hook"]
    sys.modules["antenv.axon_hooks"] = mod
    try:
        import antenv

        antenv.axon_hooks = mod
    except ImportError:
        pass

    so_path = "/opt/axon/libaxon_pjrt.so"
    if not os.path.exists(so_path):
        return
    try:
        lib = ctypes.CDLL(so_path)
    except OSError:
        return
    if not hasattr(lib, "axon_start_nrt_profile"):
        return
    lib.axon_start_nrt_profile.argtypes = [
        ctypes.POINTER(ctypes.c_int64),
        ctypes.c_size_t,
    ]
    lib.axon_start_nrt_profile.restype = ctypes.c_int64
    lib.axon_stop_nrt_profile.argtypes = [ctypes.c_char_p]
    lib.axon_stop_nrt_profile.restype = ctypes.c_int64

    @contextlib.contextmanager
    def _hook(output_dir, device_ids):
        import jax

        jax.devices()
        if device_ids:
            ids = (ctypes.c_int64 * len(device_ids))(*device_ids)
            rc = lib.axon_start_nrt_profile(ids, len(device_ids))
        else:
            rc = lib.axon_start_nrt_profile(None, 0)
        if rc != 0:
            raise RuntimeError(f"axon_start_nrt_profile rc={rc}")
        try:
            yield
        finally:
            n = lib.axon_stop_nrt_profile(str(output_dir).encode())
            print(f"ntff profile: {n} file(s) written to {output_dir}")

    mod.set_axon_ntff_profile_hook(_hook)


def _build_program():
    import concourse.tile as tile
    from concourse import bacc, mybir

    f32 = mybir.dt.float32
    bf16 = mybir.dt.bfloat16
    Exp = mybir.ActivationFunctionType.Exp

    nc = bacc.Bacc("TRN2", target_bir_lowering=False, debug=False)
    xT = nc.dram_tensor("xT", [DIM, N], bf16, kind="ExternalInput").ap()
    wq = nc.dram_tensor("wq", [DIM, HG * DHEAD], bf16, kind="ExternalInput").ap()
    wk = nc.dram_tensor("wk", [DIM, HG * DHEAD], bf16, kind="ExternalInput").ap()
    wv = nc.dram_tensor("wv", [DIM, HG * DHEAD], bf16, kind="ExternalInput").ap()
    wo = nc.dram_tensor("wo", [HG * DHEAD, DIM], bf16, kind="ExternalInput").ap()
    out = nc.dram_tensor("out", [N, DIM], bf16, kind="ExternalOutput").ap()

    with tile.TileContext(nc) as tc:
        _emit(tc, nc, mybir, out, xT, wq, wk, wv, wo, f32, bf16, Exp)
    nc.compile()
    return nc


def _emit(tc, nc, mybir, out, xT, wq, wk, wv, wo, f32, bf16, Exp):
    with (
        tc.tile_pool(name="cpool", bufs=1) as cpool,
        tc.tile_pool(name="ppool", bufs=4, space="PSUM") as ppool,
        tc.tile_pool(name="dpool", bufs=2, space="PSUM") as dpool,
        tc.tile_pool(name="epool", bufs=2) as epool,
        tc.tile_pool(name="wpool", bufs=2) as wpool,
        tc.tile_pool(name="opool", bufs=3) as opool,
        tc.tile_pool(name="drpool", bufs=2, space="DRAM") as drpool,
    ):
        # ---- constants. DMA order matters: V-projection work is first, so
        # wv and the first x^T column-chunk lead both queues; wq/wk/wo are
        # needed only ~35us in.
        wv_sb = cpool.tile([P, CT, 256], bf16, name="wv_sb")
        nc.sync.dma_start(wv_sb[:], wv.rearrange("(t p) m -> p t m", p=P))
        xt = cpool.tile([P, CT, N], bf16, name="xt")
        wq_sb = cpool.tile([P, CT, 256], bf16, name="wq_sb")
        wk_sb = cpool.tile([P, CT, 256], bf16, name="wk_sb")
        wo_sb = cpool.tile([P, 2, DIM], bf16, name="wo_sb")

        def _xt_cc(cc):
            for t in range(CT):
                eng = (nc.sync, nc.scalar)[t % 2]
                eng.dma_start(
                    xt[:, t, cc * NI : (cc + 1) * NI],
                    xT[t * P : (t + 1) * P, cc * NI : (cc + 1) * NI],
                )

        _xt_cc(0)
        nc.scalar.dma_start(wq_sb[:], wq.rearrange("(t p) m -> p t m", p=P))
        nc.scalar.dma_start(wk_sb[:], wk.rearrange("(t p) m -> p t m", p=P))
        nc.scalar.dma_start(wo_sb[:], wo.rearrange("(t p) m -> p t m", p=P))
        for cc in range(1, 4):
            _xt_cc(cc)

        # Q^T, K^T [256, N] as 2 partition-tiles; V padded to 128 cols per
        # head: [v(64) | ones(1) | zeros(63)] so lhsT is 128 wide (FWL).
        QT = cpool.tile([P, 2, N], bf16, name="QT")
        KT = cpool.tile([P, 2, N], bf16, name="KT")
        Vo = cpool.tile([P, NJT, HG * 65], bf16, name="Vo")
        Vo_heads = Vo.rearrange("p j (h c) -> p j h c", c=65)
        nc.vector.memset(Vo_heads[:, :, :, 64:65], 1.0)
        ones_sb = cpool.tile([P, 64], bf16, name="ones_sb")
        nc.vector.memset(ones_sb[:], 1.0)

        # ---- emission helpers ----
        def emit_proj_qk(pt, ic):
            q_ps = ppool.tile([P, NI], f32, tag="acc", name="q_ps")
            for ct in range(CT):
                nc.tensor.matmul(
                    q_ps,
                    lhsT=wq_sb[:, ct, pt * P : (pt + 1) * P],
                    rhs=xt[:, ct, ic * NI : (ic + 1) * NI],
                    start=(ct == 0),
                    stop=(ct == CT - 1),
                )
            nc.vector.tensor_copy(out=QT[:, pt, ic * NI : (ic + 1) * NI], in_=q_ps)
            k_ps = ppool.tile([P, NI], f32, tag="acc", name="k_ps")
            for ct in range(CT):
                nc.tensor.matmul(
                    k_ps,
                    lhsT=wk_sb[:, ct, pt * P : (pt + 1) * P],
                    rhs=xt[:, ct, ic * NI : (ic + 1) * NI],
                    start=(ct == 0),
                    stop=(ct == CT - 1),
                )
            nc.vector.tensor_copy(out=KT[:, pt, ic * NI : (ic + 1) * NI], in_=k_ps)

        def emit_proj_v(jt):
            v_ps = ppool.tile([P, 256], f32, tag="acc", name="v_ps")
            for ct in range(CT):
                nc.tensor.matmul(
                    v_ps,
                    lhsT=xt[:, ct, jt * P : (jt + 1) * P],
                    rhs=wv_sb[:, ct, :],
                    start=(ct == 0),
                    stop=(ct == CT - 1),
                )
            nc.vector.tensor_copy(
                out=Vo_heads[:, jt, :, 0:64],
                in_=v_ps.rearrange("p (h c) -> p h c", h=HG),
            )

        def emit_chunk_mms(pt, ic):
            """dots -> exp -> E@V accumulation for one (head-pair, i-chunk)."""
            hA, hB = 2 * pt, 2 * pt + 1
            E = epool.tile([P, 2, NJT, NI], bf16, tag="E", name="E")
            otA = ppool.tile([65, NI], f32, tag="acc", name="otA")
            otB = ppool.tile([65, NI], f32, tag="acc", name="otB")

            def emit_ot(jt):
                nc.tensor.matmul(
                    otA,
                    lhsT=Vo_heads[:, jt, hA, :],
                    rhs=E[:, 0, jt, :],
                    start=(jt == 0),
                    stop=(jt == NJT - 1),
                )
                nc.tensor.matmul(
                    otB,
                    lhsT=Vo_heads[:, jt, hB, :],
                    rhs=E[:, 1, jt, :],
                    start=(jt == 0),
                    stop=(jt == NJT - 1),
                )

            # software-pipelined: E@V for jt-1 is emitted after dots for jt,
            # so the PE never stalls on the exp of the tile it just produced
            for jt in range(NJT):
                dAB = dpool.tile([P, 2, NI], f32, tag="dAB", name="dAB")
                nc.tensor.matmul(
                    dAB[:, 0, :],
                    lhsT=KT[0:64, pt, jt * P : (jt + 1) * P],
                    rhs=QT[0:64, pt, ic * NI : (ic + 1) * NI],
                    start=True,
                    stop=True,
                )
                nc.tensor.matmul(
                    dAB[:, 1, :],
                    lhsT=KT[64:128, pt, jt * P : (jt + 1) * P],
                    rhs=QT[64:128, pt, ic * NI : (ic + 1) * NI],
                    start=True,
                    stop=True,
                )
                nc.scalar.activation(
                    out=E[:, :, jt, :], in_=dAB[:], func=Exp, scale=SCALE
                )
                if jt > 0:
                    emit_ot(jt - 1)
            emit_ot(NJT - 1)
            return otA, otB

        onp_tiles = {}

        def emit_norm_front(pt, ic, otA, otB):
            """DVE-only part: drain Z/O out of PSUM (frees acc slots) and
            compute 1/Z. Returns state for emit_norm_back."""
            zrow = wpool.tile([65, 2 * NI], f32, tag="zrow", name="zrow")
            zi32 = wpool.tile([65, 2 * NI], f32, tag="zi32", name="zi32")
            zi = wpool.tile([65, 2 * NI], bf16, tag="zi", name="zi")
            nc.vector.tensor_copy(out=zrow[64:65, 0:NI], in_=otA[64:65, :])
            tmpA = wpool.tile([64, NI], bf16, tag="tmp", name="tmpA", bufs=4)
            nc.vector.tensor_copy(out=tmpA, in_=otA[0:64, :])
            nc.vector.tensor_copy(out=zrow[64:65, NI : 2 * NI], in_=otB[64:65, :])
            tmpB = wpool.tile([64, NI], bf16, tag="tmp", name="tmpB", bufs=4)
            nc.vector.tensor_copy(out=tmpB, in_=otB[0:64, :])
            nc.vector.reciprocal(zi32[64:65, :], zrow[64:65, :])
            nc.vector.tensor_copy(out=zi[64:65, :], in_=zi32[64:65, :])
            # hop 1/Z to partition 0 (partition_broadcast reads partition 0)
            zi0 = wpool.tile([1, 2 * NI], bf16, tag="zi0", name="zi0", bufs=2)
            nc.sync.dma_start(zi0[:], zi[64:65, :])
            return (pt, ic, zi0, tmpA, tmpB)

        def emit_norm_back(st):
            """PE broadcast of 1/Z + the normalization multiplies."""
            pt, ic, zi0, tmpA, tmpB = st
            onp = wpool.tile([P, NI], bf16, tag=f"onp{pt}_{ic}", name="onp")
            zbA = wpool.tile([64, NI], bf16, tag="zbb", name="zbA", bufs=2)
            nc.gpsimd.partition_broadcast(zbA[:], zi0[0:1, 0:NI])
            nc.vector.tensor_mul(out=onp[0:64, :], in0=tmpA, in1=zbA)
            zbB = wpool.tile([64, NI], bf16, tag="zbb", name="zbB", bufs=2)
            nc.gpsimd.partition_broadcast(zbB[:], zi0[0:1, NI : 2 * NI])
            nbuf = wpool.tile([64, NI], bf16, tag="nbuf", name="nbuf")
            nc.vector.tensor_mul(out=nbuf, in0=tmpB, in1=zbB)
            # cross-partition move (rows 0-63 -> 64-127) via DMA
            nc.sync.dma_start(onp[64:128, :], nbuf)
            onp_tiles[(pt, ic)] = onp

        def emit_outproj(ic):
            for it_in in range(NI // P):
                for ec in range(2):
                    o_ps = ppool.tile([P, NI], f32, tag="acc", name="o_ps")
                    for pt in range(2):
                        nc.tensor.matmul(
                            o_ps,
                            lhsT=onp_tiles[(pt, ic)][:, it_in * P : (it_in + 1) * P],
                            rhs=wo_sb[:, pt, ec * NI : (ec + 1) * NI],
                            start=(pt == 0),
                            stop=(pt == 1),
                        )
                    osb = opool.tile([P, NI], bf16, tag="osb", name="osb")
                    nc.vector.tensor_copy(out=osb, in_=o_ps)
                    it = ic * (NI // P) + it_in
                    eng = (nc.sync, nc.gpsimd)[(it_in + ec) % 2]
                    eng.dma_start(
                        out[it * P : (it + 1) * P, ec * NI : (ec + 1) * NI], osb
                    )

        # ---- schedule ----
        # V and pair-0 Q/K first; pair-1 Q/K interleaved between pair-0
        # chunks; norm deferred one chunk so PE never waits on the (DVE/DMA)
        # normalization chain; outproj(ic) once both pairs of ic are normed.
        for jt in range(NJT):
            emit_proj_v(jt)
        for ic in range(NIC):
            emit_proj_qk(0, ic)

        chunks = [(0, ic) for ic in range(NIC)] + [(1, ic) for ic in range(NIC)]
        pending = None  # (pt, ic, otA, otB) awaiting norm front
        back_queue = []  # norm states awaiting the zb/mul back-half
        outproj_queue = []  # ics with both pairs normed, outproj not emitted
        for n, (pt, ic) in enumerate(chunks):
            otA, otB = emit_chunk_mms(pt, ic)
            if n < NIC:
                emit_proj_qk(1, n)
            if pending is not None:
                ppt, pic, pA, pB = pending
                st = emit_norm_front(ppt, pic, pA, pB)
                # back follows its front immediately; the whole chain then
                # completes during the next chunk, and outproj (one chunk
                # later) never waits on it
                emit_norm_back(st)
                if st[0] == 1:
                    outproj_queue.append(st[1])
            while len(outproj_queue) > 1:
                emit_outproj(outproj_queue.pop(0))
            pending = (pt, ic, otA, otB)
        ppt, pic, pA, pB = pending
        st = emit_norm_front(ppt, pic, pA, pB)
        emit_norm_back(st)
        if st[0] == 1:
            outproj_queue.append(st[1])
        while outproj_queue:
            emit_outproj(outproj_queue.pop(0))


def _get_program():
    global _CACHED_NC
    if _CACHED_NC is None:
        _CACHED_NC = _build_program()
    return _CACHED_NC


def _shard_inputs(x, Wq, Wkv, Wo):
    bf = ml_dtypes.bfloat16
    xTs = [np.ascontiguousarray(x[b].T).astype(bf) for b in range(B)]
    wqs, wks, wvs, wos = [], [], [], []
    for hg in range(HG):
        c0 = hg * HG * DHEAD
        c1 = c0 + HG * DHEAD
        wqs.append(np.ascontiguousarray(Wq[:, c0:c1]).astype(bf))
        wks.append(np.ascontiguousarray(Wkv[:, c0:c1]).astype(bf))
        wvs.append(np.ascontiguousarray(Wkv[:, DIM + c0 : DIM + c1]).astype(bf))
        wos.append(np.ascontiguousarray(Wo[c0:c1, :]).astype(bf))
    in_maps = []
    for core in range(NCORES):
        b, hg = core // HG, core % HG
        in_maps.append(
            {
                "xT": xTs[b],
                "wq": wqs[hg],
                "wk": wks[hg],
                "wv": wvs[hg],
                "wo": wos[hg],
            }
        )
    return in_maps


def kernel(x, similarity, Wq, Wkv, Wo, bo):
    global LAST_RESULTS
    _ensure_profile_hook()
    import concourse.bass_utils as _bu
    from concourse.bass_utils import run_bass_kernel_spmd

    # keep trace artifacts local if profiling is ever enabled (no S3 here)
    _bu.upload_artifacts = lambda tmpdir: tmpdir

    x = np.asarray(x, dtype=np.float32)
    Wq = np.asarray(Wq, dtype=np.float32)
    Wkv = np.asarray(Wkv, dtype=np.float32)
    Wo = np.asarray(Wo, dtype=np.float32)
    bo = np.asarray(bo, dtype=np.float32)

    nc = _get_program()
    in_maps = _shard_inputs(x, Wq, Wkv, Wo)
    res = run_bass_kernel_spmd(nc, in_maps, list(range(NCORES)))
    LAST_RESULTS = res
    outs = [res.results[i]["out"] for i in range(NCORES)]
    full = np.empty((B, N, DIM), dtype=np.float32)
    for b in range(B):
        acc = outs[4 * b].astype(np.float32).copy()
        for hg in range(1, HG):
            acc += outs[4 * b + hg]
        full[b] = acc + bo[None, :]
    return full


def _sim_check():
    """Simulate core 0 on CoreSim and compare against numpy reference."""
    from concourse.bass_interp import CoreSim

    rng = np.random.default_rng(0)
    x = rng.standard_normal((B, N, DIM), dtype=np.float32)
    Wq = (rng.standard_normal((DIM, DIM), dtype=np.float32) * DIM**-0.5).astype(
        np.float32
    )
    Wkv = (
        rng.standard_normal((DIM, 2 * DIM), dtype=np.float32) * DIM**-0.5
    ).astype(np.float32)
    Wo = (rng.standard_normal((DIM, DIM), dtype=np.float32) * DIM**-0.5).astype(
        np.float32
    )

    nc = _get_program()
    in_maps = _shard_inputs(x, Wq, Wkv, Wo)
    core = 0
    sim = CoreSim(nc)
    for name, arr in in_maps[core].items():
        sim.tensor(name)[:] = arr
    sim.simulate()
    got = np.array(sim.tensor("out"))

    # numpy reference for core 0's partial (batch 0, heads 0-3), fp32 exact
    b, hg = 0, 0
    xb = x[b]
    q = xb @ Wq[:, hg * 256 : hg * 256 + 256]
    k = xb @ Wkv[:, hg * 256 : hg * 256 + 256]
    v = xb @ Wkv[:, DIM + hg * 256 : DIM + hg * 256 + 256]
    partial = np.zeros((N, DIM), dtype=np.float32)
    for h in range(HG):
        qh = q[:, h * 64 : h * 64 + 64]
        kh = k[:, h * 64 : h * 64 + 64]
        vh = v[:, h * 64 : h * 64 + 64]
        dots = (qh @ kh.T) * SCALE
        dots -= dots.max(axis=-1, keepdims=True)
        e = np.exp(dots)
        attn = e / e.sum(axis=-1, keepdims=True)
        partial += (attn @ vh) @ Wo[hg * 256 + h * 64 : hg * 256 + h * 64 + 64, :]

    err = np.abs(got - partial)
    scale = np.abs(partial).max()
    print("max abs err:", err.max(), "scale:", scale, "rel:", err.max() / scale)
    return err.max() / scale


if __name__ == "__main__":
    _sim_check()


# revision 63
# speedup vs baseline: 1.0342x; 1.0151x over previous
"""Trainium2 Bass kernel for multi-head self-attention (B=2, N=2048, DIM=1024,
16 heads x 64). Sharding: core i handles batch b=i//4 and 4 heads hg=i%4
(tensor-parallel on heads: column-shard Wq/Wkv, row-shard Wo; partial outputs
summed on host).

Math notes:
  - `similarity` adds a per-query constant along the softmax axis, so softmax
    is invariant to it -> it is accepted but unused.
  - Softmax computed without max-subtraction (logits are O(10); exp is safe in
    fp32) as exp(dots)/Z with Z obtained for free as a 65th "ones" column of V
    in the E@V matmul.
  - Everything is computed transposed (q^T, k^T laid out [d, n]) so no
    on-device transposes are needed anywhere.
"""

import os
import sys

import numpy as np

sys.path.insert(0, "/opt/trn_rl_repo")

import ml_dtypes

B, N, DIM = 2, 2048, 1024
HEADS, DHEAD = 16, 64
HG = 4  # heads per core
SCALE = DHEAD**-0.5
NCORES = 8
P = 128
NI = 512  # i-chunk (matmul moving free dim)
NIC = N // NI  # 4 i-chunks
NJT = N // P  # 16 j tiles
CT = DIM // P  # 8 contraction tiles
GJ = 3  # j-tiles per dots psum staging group

LAST_RESULTS = None
_CACHED_NC = None


def _ensure_profile_hook():
    """Provide antenv.axon_hooks (absent in this image) so that
    run_bass_kernel_spmd(trace=True) can NTFF-profile through axon."""
    import contextlib
    import ctypes
    import types

    try:
        import antenv.axon_hooks  # noqa: F401

        return
    except ImportError:
        pass
    if "antenv.axon_hooks" in sys.modules:
        return
    mod = types.ModuleType("antenv.axon_hooks")
    state = {"hook": None}
    mod.set_axon_ntff_profile_hook = lambda h: state.__setitem__("hook", h)
    mod.get_axon_ntff_profile_hook = lambda: state["hook"]
    sys.modules["antenv.axon_hooks"] = mod
    try:
        import antenv

        antenv.axon_hooks = mod
    except ImportError:
        pass

    so_path = "/opt/axon/libaxon_pjrt.so"
    if not os.path.exists(so_path):
        return
    try:
        lib = ctypes.CDLL(so_path)
    except OSError:
        return
    if not hasattr(lib, "axon_start_nrt_profile"):
        return
    lib.axon_start_nrt_profile.argtypes = [
        ctypes.POINTER(ctypes.c_int64),
        ctypes.c_size_t,
    ]
    lib.axon_start_nrt_profile.restype = ctypes.c_int64
    lib.axon_stop_nrt_profile.argtypes = [ctypes.c_char_p]
    lib.axon_stop_nrt_profile.restype = ctypes.c_int64

    @contextlib.contextmanager
    def _hook(output_dir, device_ids):
        import jax

        jax.devices()
        if device_ids:
            ids = (ctypes.c_int64 * len(device_ids))(*device_ids)
            rc = lib.axon_start_nrt_profile(ids, len(device_ids))
        else:
            rc = lib.axon_start_nrt_profile(None, 0)
        if rc != 0:
            raise RuntimeError(f"axon_start_nrt_profile rc={rc}")
        try:
            yield
        finally:
            n = lib.axon_stop_nrt_profile(str(output_dir).encode())
            print(f"ntff profile: {n} file(s) written to {output_dir}")

    mod.set_axon_ntff_profile_hook(_hook)


def _build_program():
    import concourse.tile as tile
    from concourse import bacc, mybir

    f32 = mybir.dt.float32
    bf16 = mybir.dt.bfloat16
    Exp = mybir.ActivationFunctionType.Exp

    nc = bacc.Bacc("TRN2", target_bir_lowering=False, debug=False)
    xT = nc.dram_tensor("xT", [DIM, N], bf16, kind="ExternalInput").ap()
    wq = nc.dram_tensor("wq", [DIM, HG * DHEAD], bf16, kind="ExternalInput").ap()
    wk = nc.dram_tensor("wk", [DIM, HG * DHEAD], bf16, kind="ExternalInput").ap()
    wv = nc.dram_tensor("wv", [DIM, HG * DHEAD], bf16, kind="ExternalInput").ap()
    wo = nc.dram_tensor("wo", [HG * DHEAD, DIM], bf16, kind="ExternalInput").ap()
    out = nc.dram_tensor("out", [N, DIM], bf16, kind="ExternalOutput").ap()

    with tile.TileContext(nc) as tc:
        _emit(tc, nc, mybir, out, xT, wq, wk, wv, wo, f32, bf16, Exp)
    nc.compile()
    return nc


def _emit(tc, nc, mybir, out, xT, wq, wk, wv, wo, f32, bf16, Exp):
    with (
        tc.tile_pool(name="cpool", bufs=1) as cpool,
        tc.tile_pool(name="ppool", bufs=4, space="PSUM") as ppool,
        tc.tile_pool(name="dpool", bufs=2, space="PSUM") as dpool,
        tc.tile_pool(name="epool", bufs=2) as epool,
        tc.tile_pool(name="wpool", bufs=2) as wpool,
        tc.tile_pool(name="opool", bufs=3) as opool,
        tc.tile_pool(name="drpool", bufs=2, space="DRAM") as drpool,
    ):
        # ---- constants. DMA order matters: V-projection work is first, so
        # wv and the first x^T column-chunk lead both queues; wq/wk/wo are
        # needed only ~35us in.
        wv_sb = cpool.tile([P, CT, 256], bf16, name="wv_sb")
        nc.sync.dma_start(wv_sb[:], wv.rearrange("(t p) m -> p t m", p=P))
        xt = cpool.tile([P, CT, N], bf16, name="xt")
        wq_sb = cpool.tile([P, CT, 256], bf16, name="wq_sb")
        wk_sb = cpool.tile([P, CT, 256], bf16, name="wk_sb")
        wo_sb = cpool.tile([P, 2, DIM], bf16, name="wo_sb")

        def _xt_cc(cc):
            for t in range(CT):
                eng = (nc.sync, nc.scalar)[t % 2]
                eng.dma_start(
                    xt[:, t, cc * NI : (cc + 1) * NI],
                    xT[t * P : (t + 1) * P, cc * NI : (cc + 1) * NI],
                )

        _xt_cc(0)
        nc.scalar.dma_start(wq_sb[:], wq.rearrange("(t p) m -> p t m", p=P))
        nc.scalar.dma_start(wk_sb[:], wk.rearrange("(t p) m -> p t m", p=P))
        nc.scalar.dma_start(wo_sb[:], wo.rearrange("(t p) m -> p t m", p=P))
        for cc in range(1, 4):
            _xt_cc(cc)

        # Q^T, K^T [256, N] as 2 partition-tiles; V padded to 128 cols per
        # head: [v(64) | ones(1) | zeros(63)] so lhsT is 128 wide (FWL).
        QT = cpool.tile([P, 2, N], bf16, name="QT")
        KT = cpool.tile([P, 2, N], bf16, name="KT")
        Vo = cpool.tile([P, NJT, HG * 65], bf16, name="Vo")
        Vo_heads = Vo.rearrange("p j (h c) -> p j h c", c=65)
        nc.vector.memset(Vo_heads[:, :, :, 64:65], 1.0)
        ones_sb = cpool.tile([P, 64], bf16, name="ones_sb")
        nc.vector.memset(ones_sb[:], 1.0)

        # ---- emission helpers ----
        def emit_proj_qk(pt, ic):
            q_ps = ppool.tile([P, NI], f32, tag="acc", name="q_ps")
            for ct in range(CT):
                nc.tensor.matmul(
                    q_ps,
                    lhsT=wq_sb[:, ct, pt * P : (pt + 1) * P],
                    rhs=xt[:, ct, ic * NI : (ic + 1) * NI],
                    start=(ct == 0),
                    stop=(ct == CT - 1),
                )
            nc.vector.tensor_copy(out=QT[:, pt, ic * NI : (ic + 1) * NI], in_=q_ps)
            k_ps = ppool.tile([P, NI], f32, tag="acc", name="k_ps")
            for ct in range(CT):
                nc.tensor.matmul(
                    k_ps,
                    lhsT=wk_sb[:, ct, pt * P : (pt + 1) * P],
                    rhs=xt[:, ct, ic * NI : (ic + 1) * NI],
                    start=(ct == 0),
                    stop=(ct == CT - 1),
                )
            nc.vector.tensor_copy(out=KT[:, pt, ic * NI : (ic + 1) * NI], in_=k_ps)

        def emit_proj_v(jt):
            v_ps = ppool.tile([P, 256], f32, tag="acc", name="v_ps")
            for ct in range(CT):
                nc.tensor.matmul(
                    v_ps,
                    lhsT=xt[:, ct, jt * P : (jt + 1) * P],
                    rhs=wv_sb[:, ct, :],
                    start=(ct == 0),
                    stop=(ct == CT - 1),
                )
            nc.vector.tensor_copy(
                out=Vo_heads[:, jt, :, 0:64],
                in_=v_ps.rearrange("p (h c) -> p h c", h=HG),
            )

        def emit_chunk_mms(pt, ic):
            """dots -> exp -> E@V accumulation for one (head-pair, i-chunk)."""
            hA, hB = 2 * pt, 2 * pt + 1
            E = epool.tile([P, 2, NJT, NI], bf16, tag="E", name="E")
            otA = ppool.tile([65, NI], f32, tag="acc", name="otA")
            otB = ppool.tile([65, NI], f32, tag="acc", name="otB")

            def emit_ot(jt):
                nc.tensor.matmul(
                    otA,
                    lhsT=Vo_heads[:, jt, hA, :],
                    rhs=E[:, 0, jt, :],
                    start=(jt == 0),
                    stop=(jt == NJT - 1),
                )
                nc.tensor.matmul(
                    otB,
                    lhsT=Vo_heads[:, jt, hB, :],
                    rhs=E[:, 1, jt, :],
                    start=(jt == 0),
                    stop=(jt == NJT - 1),
                )

            # software-pipelined: E@V for jt-1 is emitted after dots for jt,
            # so the PE never stalls on the exp of the tile it just produced
            for jt in range(NJT):
                dAB = dpool.tile([P, 2, NI], f32, tag="dAB", name="dAB")
                nc.tensor.matmul(
                    dAB[:, 0, :],
                    lhsT=KT[0:64, pt, jt * P : (jt + 1) * P],
                    rhs=QT[0:64, pt, ic * NI : (ic + 1) * NI],
                    start=True,
                    stop=True,
                )
                nc.tensor.matmul(
                    dAB[:, 1, :],
                    lhsT=KT[64:128, pt, jt * P : (jt + 1) * P],
                    rhs=QT[64:128, pt, ic * NI : (ic + 1) * NI],
                    start=True,
                    stop=True,
                )
                nc.scalar.activation(
                    out=E[:, :, jt, :], in_=dAB[:], func=Exp, scale=SCALE
                )
                if jt > 0:
                    emit_ot(jt - 1)
            emit_ot(NJT - 1)
            return otA, otB

        onp_tiles = {}

        def emit_norm_front(pt, ic, otA, otB):
            """DVE-only part: drain Z/O out of PSUM (frees acc slots) and
            compute 1/Z. Returns state for emit_norm_back."""
            zrow = wpool.tile([65, 2 * NI], f32, tag="zrow", name="zrow")
            zi32 = wpool.tile([65, 2 * NI], f32, tag="zi32", name="zi32")
            zi = wpool.tile([65, 2 * NI], bf16, tag="zi", name="zi")
            nc.vector.tensor_copy(out=zrow[64:65, 0:NI], in_=otA[64:65, :])
            tmpA = wpool.tile([64, NI], bf16, tag="tmp", name="tmpA", bufs=4)
            nc.vector.tensor_copy(out=tmpA, in_=otA[0:64, :])
            nc.vector.tensor_copy(out=zrow[64:65, NI : 2 * NI], in_=otB[64:65, :])
            tmpB = wpool.tile([64, NI], bf16, tag="tmp", name="tmpB", bufs=4)
            nc.vector.tensor_copy(out=tmpB, in_=otB[0:64, :])
            nc.vector.reciprocal(zi32[64:65, :], zrow[64:65, :])
            nc.vector.tensor_copy(out=zi[64:65, :], in_=zi32[64:65, :])
            # hop 1/Z to partition 0 (partition_broadcast reads partition 0)
            zi0 = wpool.tile([1, 2 * NI], bf16, tag="zi0", name="zi0", bufs=2)
            nc.sync.dma_start(zi0[:], zi[64:65, :])
            return (pt, ic, zi0, tmpA, tmpB)

        def emit_norm_back(st):
            """PE broadcast of 1/Z + the normalization multiplies."""
            pt, ic, zi0, tmpA, tmpB = st
            onp = wpool.tile([P, NI], bf16, tag=f"onp{pt}_{ic}", name="onp")
            zbA = wpool.tile([64, NI], bf16, tag="zbb", name="zbA", bufs=2)
            nc.gpsimd.partition_broadcast(zbA[:], zi0[0:1, 0:NI])
            nc.vector.tensor_mul(out=onp[0:64, :], in0=tmpA, in1=zbA)
            zbB = wpool.tile([64, NI], bf16, tag="zbb", name="zbB", bufs=2)
            nc.gpsimd.partition_broadcast(zbB[:], zi0[0:1, NI : 2 * NI])
            nbuf = wpool.tile([64, NI], bf16, tag="nbuf", name="nbuf")
            nc.vector.tensor_mul(out=nbuf, in0=tmpB, in1=zbB)
            # cross-partition move (rows 0-63 -> 64-127) via DMA
            nc.sync.dma_start(onp[64:128, :], nbuf)
            onp_tiles[(pt, ic)] = onp

        def emit_outproj(ic):
            for it_in in range(NI // P):
                for ec in range(2):
                    o_ps = ppool.tile([P, NI], f32, tag="acc", name="o_ps")
                    for pt in range(2):
                        nc.tensor.matmul(
                            o_ps,
                            lhsT=onp_tiles[(pt, ic)][:, it_in * P : (it_in + 1) * P],
                            rhs=wo_sb[:, pt, ec * NI : (ec + 1) * NI],
                            start=(pt == 0),
                            stop=(pt == 1),
                        )
                    osb = opool.tile([P, NI], bf16, tag="osb", name="osb")
                    nc.vector.tensor_copy(out=osb, in_=o_ps)
                    it = ic * (NI // P) + it_in
                    eng = (nc.sync, nc.gpsimd)[(it_in + ec) % 2]
                    eng.dma_start(
                        out[it * P : (it + 1) * P, ec * NI : (ec + 1) * NI], osb
                    )

        # ---- schedule ----
        # V and pair-0 Q/K first; pair-1 Q/K interleaved between pair-0
        # chunks; norm deferred one chunk so PE never waits on the (DVE/DMA)
        # normalization chain; outproj(ic) once both pairs of ic are normed.
        for jt in range(NJT):
            emit_proj_v(jt)
        for ic in range(NIC):
            emit_proj_qk(0, ic)

        chunks = [(0, ic) for ic in range(NIC)] + [(1, ic) for ic in range(NIC)]
        pending = None  # (pt, ic, otA, otB) awaiting norm front
        back_queue = []  # norm states awaiting the zb/mul back-half
        outproj_queue = []  # ics with both pairs normed, outproj not emitted
        for n, (pt, ic) in enumerate(chunks):
            otA, otB = emit_chunk_mms(pt, ic)
            if n < NIC:
                emit_proj_qk(1, n)
            if pending is not None:
                ppt, pic, pA, pB = pending
                st = emit_norm_front(ppt, pic, pA, pB)
                # back follows its front immediately; the whole chain then
                # completes during the next chunk, and outproj (one chunk
                # later) never waits on it
                emit_norm_back(st)
                if st[0] == 1:
                    outproj_queue.append(st[1])
            while len(outproj_queue) > 1:
                emit_outproj(outproj_queue.pop(0))
            pending = (pt, ic, otA, otB)
        ppt, pic, pA, pB = pending
        st = emit_norm_front(ppt, pic, pA, pB)
        emit_norm_back(st)
        if st[0] == 1:
            outproj_queue.append(st[1])
        while outproj_queue:
            emit_outproj(outproj_queue.pop(0))


def _get_program():
    global _CACHED_NC
    if _CACHED_NC is None:
        _CACHED_NC = _build_program()
    return _CACHED_NC


def _shard_inputs(x, Wq, Wkv, Wo):
    bf = ml_dtypes.bfloat16
    xTs = [np.ascontiguousarray(x[b].T).astype(bf) for b in range(B)]
    wqs, wks, wvs, wos = [], [], [], []
    for hg in range(HG):
        c0 = hg * HG * DHEAD
        c1 = c0 + HG * DHEAD
        wqs.append(np.ascontiguousarray(Wq[:, c0:c1]).astype(bf))
        wks.append(np.ascontiguousarray(Wkv[:, c0:c1]).astype(bf))
        wvs.append(np.ascontiguousarray(Wkv[:, DIM + c0 : DIM + c1]).astype(bf))
        wos.append(np.ascontiguousarray(Wo[c0:c1, :]).astype(bf))
    in_maps = []
    for core in range(NCORES):
        b, hg = core // HG, core % HG
        in_maps.append(
            {
                "xT": xTs[b],
                "wq": wqs[hg],
                "wk": wks[hg],
                "wv": wvs[hg],
                "wo": wos[hg],
            }
        )
    return in_maps


def kernel(x, similarity, Wq, Wkv, Wo, bo):
    global LAST_RESULTS
    _ensure_profile_hook()
    import concourse.bass_utils as _bu
    from concourse.bass_utils import run_bass_kernel_spmd

    # keep trace artifacts local if profiling is ever enabled (no S3 here)
    _bu.upload_artifacts = lambda tmpdir: tmpdir

    x = np.asarray(x, dtype=np.float32)
    Wq = np.asarray(Wq, dtype=np.float32)
    Wkv = np.asarray(Wkv, dtype=np.float32)
    Wo = np.asarray(Wo, dtype=np.float32)
    bo = np.asarray(bo, dtype=np.float32)

    nc = _get_program()
    in_maps = _shard_inputs(x, Wq, Wkv, Wo)
    res = run_bass_kernel_spmd(nc, in_maps, list(range(NCORES)))
    LAST_RESULTS = res
    outs = [res.results[i]["out"] for i in range(NCORES)]
    full = np.empty((B, N, DIM), dtype=np.float32)
    for b in range(B):
        acc = outs[4 * b].astype(np.float32).copy()
        for hg in range(1, HG):
            acc += outs[4 * b + hg]
        full[b] = acc + bo[None, :]
    return full


def _sim_check():
    """Simulate core 0 on CoreSim and compare against numpy reference."""
    from concourse.bass_interp import CoreSim

    rng = np.random.default_rng(0)
    x = rng.standard_normal((B, N, DIM), dtype=np.float32)
    Wq = (rng.standard_normal((DIM, DIM), dtype=np.float32) * DIM**-0.5).astype(
        np.float32
    )
    Wkv = (
        rng.standard_normal((DIM, 2 * DIM), dtype=np.float32) * DIM**-0.5
    ).astype(np.float32)
    Wo = (rng.standard_normal((DIM, DIM), dtype=np.float32) * DIM**-0.5).astype(
        np.float32
    )

    nc = _get_program()
    in_maps = _shard_inputs(x, Wq, Wkv, Wo)
    core = 0
    sim = CoreSim(nc)
    for name, arr in in_maps[core].items():
        sim.tensor(name)[:] = arr
    sim.simulate()
    got = np.array(sim.tensor("out"))

    # numpy reference for core 0's partial (batch 0, heads 0-3), fp32 exact
    b, hg = 0, 0
    xb = x[b]
    q = xb @ Wq[:, hg * 256 : hg * 256 + 256]
    k = xb @ Wkv[:, hg * 256 : hg * 256 + 256]
    v = xb @ Wkv[:, DIM + hg * 256 : DIM + hg * 256 + 256]
    partial = np.zeros((N, DIM), dtype=np.float32)
    for h in range(HG):
        qh = q[:, h * 64 : h * 64 + 64]
        kh = k[:, h * 64 : h * 64 + 64]
        vh = v[:, h * 64 : h * 64 + 64]
        dots = (qh @ kh.T) * SCALE
        dots -= dots.max(axis=-1, keepdims=True)
        e = np.exp(dots)
        attn = e / e.sum(axis=-1, keepdims=True)
        partial += (attn @ vh) @ Wo[hg * 256 + h * 64 : hg * 256 + h * 64 + 64, :]

    err = np.abs(got - partial)
    scale = np.abs(partial).max()
    print("max abs err:", err.max(), "scale:", scale, "rel:", err.max() / scale)
    return err.max() / scale


if __name__ == "__main__":
    _sim_check()
